# revision 49
# baseline (speedup 1.0000x reference)
"""Trainium2 Bass kernel for nn_ChannelAttention (S=2048, B=8, D=1024, DH=512).

Reference semantics (jax, fp32):
    q_t = q @ Wq.T + bq   (S,B,D) -> (S,B,DH)     [same for k, v]
    q_ = q_t.reshape(B, DH, S)   # torch-style raw view of the flat buffer
    k_ = k_t.reshape(B, S, DH)
    attn = softmax(mask(q_ @ k_), -1)              # (B, DH, DH)
    out  = (attn @ v_t.reshape(B, DH, S)).reshape(S, B, DH)

The raw views make the bmm "batch" dim index contiguous 1M-element chunks of
the flat (S*B*DH) buffer = chunks of 256 consecutive s values, so sharding
over s-chunks of 256 makes everything core-local. Per core (T=2048 tokens,
D=1024, E=DH=512):
    AT[e,t] = Wq Xq^T + bq        (Q, transposed layout for bmm1 lhsT)
    B[t,e]  = Xk Wk^T + bk        (K)
    attn    = softmax(mask(Qm @ Km))   Qm/Km = strided views of AT/B
    C       = reshape(Xv Wv^T + bv)    (fp16)
    out     = attn @ C                 (bmm2, fp16 operands)

Schedule: the PE stream is ordered K0 K1 Q0 K2 Q1 K3 bmm1-0 Q2 bmm1-1 Q3
bmm1-2 bmm1-3 [V/T interleaved] bmm2-0..3 so every group's operands are
ready when the PE reaches it (softmax DVE/Act work hides under the V
projections, transposes precede their bmm2 group). DMA: the kernel is
bus-bound for its first ~45us, so loads are issued on 3 queues in exact
consumption order, with the x-tile pool ring (bufs=3, ring order == PE
order) WAR-throttling later pieces so they cannot preempt earlier ones on
the shared DMA bus; the V-side loads queue behind everything on gpsimd.
Precision: Q/K path f32r (accuracy-critical pre-softmax: output err is
~linear in q_t/k_t element error, measured 5.6e-3 vs the 2e-2 gate; fp16
here would be ~1.8e-2), V path + attn weights + bmm2 + output fp16
(post-softmax, halves those DMA bytes, no measurable error change).
Softmax normalization is folded into the bmm2 output copy (scale=1/rowsum).
Steady-state per-rep == the f32r PE floor (~110us: 266k rows at 1 row/cyc
@2.4GHz, measured 0.4157 ns/row); tried and rejected: fp8-DoubleRow
compensated V projection (slower on HW than the cost model's 0.5 cyc/row
and rel err 2.2e-2), PE warm-up dummies (gaps are operand waits, not
p-state resets).
"""

import numpy as np

import concourse.bass as bass
import concourse.mybir as mybir
import concourse.tile as tile
from concourse import bacc
from concourse.bass_utils import run_bass_kernel_spmd
from concourse.masks import make_identity

N_CORES = 8
S, B, D, DH = 2048, 8, 1024, 512
SC = S // N_CORES          # 256 s per core
T = SC * B                 # 2048 tokens per core
NEG = -49152.0  # fp8e5-representable; |logits| < 200 so this still masks to exp()=0

F32 = mybir.dt.float32
F32R = mybir.dt.float32r
F16 = mybir.dt.float16


def build_nc(reps: int = 1, use_f32r: bool = True):
    """Build + compile the per-core SPMD program. reps>1 repeats the body
    back-to-back (for wall-clock delta timing)."""
    mm_dt = F32R if use_f32r else F32
    nc = bacc.Bacc("TRN2", target_bir_lowering=False, debug=False,
                   num_devices=N_CORES)

    # DRAM I/O (per core). X/W transposed on host. Q/K side declared f32r so
    # the DMA'd bits are directly legal as f32r matmul operands; V side fp16.
    # x: (4 chunks, 128 partitions, 8 ktiles * 512 t)
    xq = nc.declare_dram_parameter("xq", [4, 128, 8 * 512], mm_dt, isOutput=False)
    xk = nc.declare_dram_parameter("xk", [4, 128, 8 * 512], mm_dt, isOutput=False)
    xv = nc.declare_dram_parameter("xv", [4, 128, 8 * 512], F16, isOutput=False)
    wq = nc.declare_dram_parameter("wq", [128, 8 * DH], mm_dt, isOutput=False)
    wk = nc.declare_dram_parameter("wk", [128, 8 * DH], mm_dt, isOutput=False)
    wv = nc.declare_dram_parameter("wv", [128, 8 * DH], F16, isOutput=False)
    bq = nc.declare_dram_parameter("bq", [DH], F32, isOutput=False)
    bk = nc.declare_dram_parameter("bk", [DH], F32, isOutput=False)
    bv = nc.declare_dram_parameter("bv", [DH], F32, isOutput=False)
    maskadd = nc.declare_dram_parameter("maskadd", [128, 4 * DH], mybir.dt.float8e5, isOutput=False)
    out = nc.declare_dram_parameter("out", [DH, T], F16, isOutput=True)

    with tile.TileContext(nc) as tc:
        with (
            tc.tile_pool(name="singles", bufs=1) as singles,
            tc.tile_pool(name="wpool", bufs=1) as wpool,
            tc.tile_pool(name="xpool", bufs=3) as xpool,
            tc.tile_pool(name="xvpool", bufs=2) as xvpool,
            tc.tile_pool(name="proj", bufs=1) as proj,
            tc.tile_pool(name="sm", bufs=2) as sm,
            tc.tile_pool(name="stat", bufs=2) as stat,
            tc.tile_pool(name="pp", bufs=4, space="PSUM") as pp,
            tc.tile_pool(name="tp", bufs=2, space="PSUM") as tp,
            tc.tile_pool(name="op", bufs=2, space="PSUM") as op,
        ):
            def dma_halves(dst, src_ap, eng):
                n = dst.shape[-1]
                half = src_ap.shape[-1] // 2
                eng.dma_start(
                    out=dst[:, 0:4, :],
                    in_=src_ap[:, 0:half].rearrange("p (k n) -> p k n", n=n))
                eng.dma_start(
                    out=dst[:, 4:8, :],
                    in_=src_ap[:, half:].rearrange("p (k n) -> p k n", n=n))

            for rep in range(reps):
                # ---- tiles ----
                wk_sb = wpool.tile([128, 8, DH], mm_dt, tag="wk")
                wq_sb = wpool.tile([128, 8, DH], mm_dt, tag="wq")
                wv_sb = wpool.tile([128, 8, DH], F16, tag="wv")
                # xq/xk share one 3-buf ring; ring/issue order chosen so the
                # pool's WAR dependencies throttle each DMA to land just
                # before its PE group (bus order ~= consumption order).
                xck, xcq = [None] * 4, [None] * 4
                ring = (("k", 0), ("k", 1), ("q", 0), ("k", 2),
                        ("q", 1), ("k", 3), ("q", 2), ("q", 3))
                for nm, ct in ring:
                    t_ = xpool.tile([128, 8, 512], mm_dt, tag="x", name=f"xc{nm}{ct}")
                    (xck if nm == "k" else xcq)[ct] = t_
                xcv = [xvpool.tile([128, 8, 512], F16, tag="xv", name=f"xcv{ct}")
                       for ct in range(4)]

                at_sb = proj.tile([128, 4, T], mm_dt, tag="at")     # [e%128, me, t]
                b_sb = proj.tile([128, 16, DH], mm_dt, tag="b")     # [t%128, t//128, e]
                c_sb = proj.tile([128, 4, 4, DH], F16, tag="c")     # [t'%128, ts, kt', e]
                p_sb = proj.tile([128, 4, DH], F16, tag="p")        # exp(logits-max)
                pt_sb = proj.tile([128, 4, DH], F16, tag="pt")      # P^T
                recips = proj.tile([128, 4], F32, tag="recips")     # 1/rowsum per mt

                # ---- DMA issue, consumption order, 3 queues ----
                # Each dma_start costs the issuing engine ~0.6-1us and each
                # DMA holds the shared bus for its duration, so piece size
                # trades startup latency against issue overhead: fine lead
                # pieces for the two tiles the first matmuls need, halves
                # elsewhere.
                def dma_lead(dst, src_ap, eng):
                    # 3 pieces: fast-ish first matmul without drip-feeding
                    # (each trigger costs ~1.2us of issue cadence per queue)
                    n = dst.shape[-1]
                    src = src_ap.rearrange("p (k n) -> p k n", n=n)
                    for lo, hi in ((0, 1), (1, 3), (3, 5), (5, 8)):
                        eng.dma_start(out=dst[:, lo:hi, :], in_=src[:, lo:hi])

                # wq behind xck1 on sync so it cannot preempt the K-phase
                # loads on the shared bus; the x ring (WAR deps, ring order ==
                # PE consumption order, bufs=3 -> 2-group DMA lead) throttles
                # everything from xck2 on to land just-in-time.
                dma_lead(wk_sb, wk.ap(), nc.sync)
                dma_lead(xck[0], xk.ap()[0], nc.gpsimd)
                dma_halves(xck[1], xk.ap()[1], nc.sync)
                dma_halves(xcq[0], xq.ap()[0], nc.gpsimd)
                dma_halves(wq_sb, wq.ap(), nc.sync)
                dma_halves(xck[2], xk.ap()[2], nc.gpsimd)
                dma_halves(xcq[1], xq.ap()[1], nc.gpsimd)
                dma_halves(xck[3], xk.ap()[3], nc.gpsimd)
                dma_halves(xcq[2], xq.ap()[2], nc.gpsimd)
                dma_halves(xcq[3], xq.ap()[3], nc.gpsimd)
                # V side at the END of the gpsimd queue: FIFO behind the
                # WAR-throttled Q pieces keeps it off the bus until the
                # projection loads are through.
                dma_halves(wv_sb, wv.ap(), nc.gpsimd)
                for ct in range(4):
                    dma_halves(xcv[ct], xv.ap()[ct], nc.gpsimd)

                bq_sb = singles.tile([128, 4], F32)
                nc.scalar.dma_start(out=bq_sb,
                                    in_=bq.ap().rearrange("(me p) -> p me", p=128))
                bk_sb = singles.tile([128, DH], F32)
                bv_sb = singles.tile([128, DH], F32)
                bk_src = bk.ap()
                nc.scalar.dma_start(out=bk_sb, in_=bass.AP(
                    tensor=bk_src.tensor, offset=bk_src.offset,
                    ap=[[0, 128], [1, DH]]))
                bv_src = bv.ap()
                nc.scalar.dma_start(out=bv_sb, in_=bass.AP(
                    tensor=bv_src.tensor, offset=bv_src.offset,
                    ap=[[0, 128], [1, DH]]))
                mask_sb = singles.tile([128, 4, DH], mybir.dt.float8e5)
                nc.scalar.dma_start(
                    out=mask_sb,
                    in_=maskadd.ap().rearrange("p (mt e) -> p mt e", mt=4))

                # NOTE: emitted here (between the mask and V-side DMA issues)
                # deliberately -- these two Pool-engine ops space the V-side
                # triggers off the Q pieces on the shared bus; hoisting them
                # out of the loop measurably hurts steady-state in the
                # timeline sim.
                identity = singles.tile([128, 128], F16, tag="identity",
                                        name=f"id{rep}")
                make_identity(nc, identity)



                # ---- PE groups ----
                def kproj(ct):
                    # B[t, e] = sum_d XkT[d, t] * WkT[d, e] + bk[e]
                    # kd-major: the half-tile DMA boundary falls between
                    # matmuls 16/17 of the group instead of dripping through
                    # every 8-chain (4 accumulators in flight).
                    accs = [pp.tile([128, DH], F32, tag="acc",
                                    name=f"ka{ct}_{mi}") for mi in range(4)]
                    for kd in range(4):
                        for mi in range(4):
                            nc.tensor.matmul(
                                accs[mi][:, :],
                                xck[ct][:, kd, 128*mi:128*(mi+1)],
                                wk_sb[:, kd, :],
                                start=(kd == 0), stop=False)
                    for mi in range(4):
                        for kd in range(4, 8):
                            nc.tensor.matmul(
                                accs[mi][:, :],
                                xck[ct][:, kd, 128*mi:128*(mi+1)],
                                wk_sb[:, kd, :],
                                start=False, stop=(kd == 7))
                        nc.vector.tensor_add(b_sb[:, 4*ct+mi, :], accs[mi][:, :], bk_sb)

                def qproj(ct):
                    # AT[e, t] = sum_d WqT[d, e] * XqT[d, t] + bq[e]
                    accs = [pp.tile([128, DH], F32, tag="acc",
                                    name=f"qa{ct}_{me}") for me in range(4)]
                    for kd in range(4):
                        for me in range(4):
                            nc.tensor.matmul(
                                accs[me][:, :],
                                wq_sb[:, kd, 128*me:128*(me+1)],
                                xcq[ct][:, kd, :],
                                start=(kd == 0), stop=False)
                    for me in range(4):
                        for kd in range(4, 8):
                            nc.tensor.matmul(
                                accs[me][:, :],
                                wq_sb[:, kd, 128*me:128*(me+1)],
                                xcq[ct][:, kd, :],
                                start=False, stop=(kd == 7))
                        nc.scalar.activation(
                            at_sb[:, me, 512*ct:512*(ct+1)], accs[me][:, :],
                            mybir.ActivationFunctionType.Identity,
                            bias=bq_sb[:, me:me+1])

                def vproj(ct):
                    # C_ts[r', e] = (Xv Wv^T + bv) in Vm layout, fp16
                    for ts in range(4):
                        acc = pp.tile([128, DH], F32, tag="acc")
                        for kd in range(8):
                            nc.tensor.matmul(
                                acc[:, :],
                                xcv[ct][:, kd, ts:ts+509:4],
                                wv_sb[:, kd, :],
                                start=(kd == 0), stop=(kd == 7))
                        nc.vector.tensor_add(c_sb[:, ts, ct, :], acc[:, :], bv_sb)

                def bmm1(mt):
                    # attn[r, r'] += Qm-tile @ Km-tile over 16 k-tiles; then
                    # mask + rowmax + exp (+rowsum) on DVE/Act; 1/rowsum saved.
                    acc = pp.tile([128, DH], F32, tag="acc")
                    for kt in range(16):
                        ts, ei = divmod(kt, 4)
                        st = 512*mt + ts
                        nc.tensor.matmul(
                            acc[:, :],
                            at_sb[:, ei, st:st+509:4],
                            b_sb[:, kt, :],
                            start=(kt == 0), stop=(kt == 15))
                    # mask-add in place on PSUM (saves an SBUF tile + copy)
                    nc.vector.tensor_add(acc[:, :], acc[:, :], mask_sb[:, mt, :])
                    negmax = stat.tile([128, 1], F32, tag="nmax")
                    nc.vector.reduce_max(negmax, acc[:, :],
                                         axis=mybir.AxisListType.X, negate=True)
                    rowsum = stat.tile([128, 1], F32, tag="rsum")
                    nc.scalar.activation(
                        p_sb[:, mt, :], acc[:, :],
                        mybir.ActivationFunctionType.Exp,
                        bias=negmax, scale=1.0, accum_out=rowsum)
                    nc.vector.reciprocal(recips[:, mt:mt+1], rowsum)

                def transp(mt):
                    # 4 transposes into one PSUM tile, then ONE strided copy
                    # into pt_sb (keeps Act off the PE critical path).
                    ptp = tp.tile([128, 4, 128], F16, tag="ptp")
                    for kt in range(4):
                        nc.tensor.transpose(ptp[:, kt, :], p_sb[:, mt, 128*kt:128*(kt+1)],
                                            identity[:, :])
                    nc.scalar.copy(pt_sb[:, :, 128*mt:128*(mt+1)], ptp[:, :, :])

                def bmm2(mt):
                    # out[r, 512*tsp+e'] = (1/rowsum[r]) * sum_r' P~[r,r'] C[r',e']
                    # 4 tsp blocks scale-copied into one SBUF row tile, single
                    # output DMA per mt (alternating queues).
                    omt = sm.tile([128, 4, DH], F16, tag="osb")
                    for tsp in range(4):
                        acc = op.tile([128, DH], F32, tag="acc2")
                        for ktp in range(4):
                            nc.tensor.matmul(
                                acc[:, :],
                                pt_sb[:, ktp, 128*mt:128*(mt+1)],
                                c_sb[:, tsp, ktp, :],
                                start=(ktp == 0), stop=(ktp == 3))
                        if tsp % 2 == 0:
                            nc.vector.tensor_scalar_mul(omt[:, tsp, :], acc[:, :],
                                                        recips[:, mt:mt+1])
                        else:
                            nc.scalar.activation(
                                omt[:, tsp, :], acc[:, :],
                                mybir.ActivationFunctionType.Copy,
                                scale=recips[:, mt:mt+1])
                    orows = out[128*mt:128*(mt+1), :].rearrange(
                        "p (ts e) -> p ts e", e=DH)
                    for tsp in range(4):
                        eng = nc.sync if (4*mt + tsp) % 2 == 0 else nc.gpsimd
                        eng.dma_start(out=orows[:, tsp:tsp+1],
                                      in_=omt[:, tsp:tsp+1, :])

                kproj(0); kproj(1); qproj(0); kproj(2); qproj(1); kproj(3)
                bmm1(0); qproj(2); bmm1(1); qproj(3); bmm1(2); bmm1(3)
                vproj(0); transp(0); vproj(1); transp(1)
                vproj(2); transp(2); vproj(3); transp(3)
                bmm2(0); bmm2(1); bmm2(2); bmm2(3)
    nc.compile()
    return nc


def make_in_maps(q, k, v, attn_mask, Wq, bq, Wk, bk, Wv, bv):
    q = np.asarray(q, dtype=np.float32)
    k = np.asarray(k, dtype=np.float32)
    v = np.asarray(v, dtype=np.float32)
    attn_mask = np.asarray(attn_mask)
    import ml_dtypes
    maskadd = np.where(attn_mask, np.float32(NEG), np.float32(0.0)).astype(np.float32)
    # pre-tile: (512, 512) -> (128, 4*512) with [p, mt*512+e] = maskadd[128*mt+p, e]
    maskadd = np.ascontiguousarray(
        maskadd.reshape(4, 128, DH).transpose(1, 0, 2).reshape(128, 4 * DH)
    ).astype(ml_dtypes.float8_e5m2)

    def prep_w(W, dt=np.float32):
        # W (DH, D) -> W.T (D, DH) -> (128, 8*512): [p, kd*512+e] = W.T[128*kd+p, e]
        wt = np.asarray(W, dtype=np.float32).T
        return np.ascontiguousarray(
            wt.reshape(8, 128, DH).transpose(1, 0, 2).reshape(128, 8 * DH)).astype(dt)

    wqt, wkt = prep_w(Wq), prep_w(Wk)
    wvt = prep_w(Wv, np.float16)

    def prep_x(x_slice, dt=np.float32):
        # (SC, B, D) -> tokens x D -> X.T (D, T) -> (4, 128, 8*512):
        # [ct, p, kd*512+t'] = X.T[128*kd+p, 512*ct+t']
        xt = x_slice.reshape(T, D).T                      # (1024, 2048)
        x4 = xt.reshape(8, 128, 4, 512)                   # [kd, p, ct, t']
        return np.ascontiguousarray(
            x4.transpose(2, 1, 0, 3).reshape(4, 128, 8 * 512)).astype(dt)
    bq = np.asarray(bq, dtype=np.float32)
    bk = np.asarray(bk, dtype=np.float32)
    bv = np.asarray(bv, dtype=np.float32)
    in_maps = []
    for c in range(N_CORES):
        sl = slice(SC * c, SC * (c + 1))
        in_maps.append({
            "xq": prep_x(q[sl]),
            "xk": prep_x(k[sl]),
            "xv": prep_x(v[sl], np.float16),
            "wq": wqt, "wk": wkt, "wv": wvt,
            "bq": bq, "bk": bk, "bv": bv,
            "maskadd": maskadd,
        })
    return in_maps


_nc_cache = {}


def kernel(q, k, v, attn_mask, Wq, bq, Wk, bk, Wv, bv):
    if "nc" not in _nc_cache:
        _nc_cache["nc"] = build_nc(reps=1)
    nc = _nc_cache["nc"]
    in_maps = make_in_maps(q, k, v, attn_mask, Wq, bq, Wk, bk, Wv, bv)
    res = run_bass_kernel_spmd(nc, in_maps, list(range(N_CORES))).results
    out = np.concatenate(
        [res[c]["out"].astype(np.float32).reshape(SC, B, DH)
         for c in range(N_CORES)], axis=0)
    return out
